# revision 22
# baseline (speedup 1.0000x reference)
"""Multi-head attention (B=2, S=2048, D=1024, H=16) on 8 NeuronCores.

Sharding: core c -> batch b = c//4, head group g = c%4 (4 heads each).
Each core computes q/k/v projections for its head group, full softmax
attention for its 4 heads, and a partial output projection
out_c = attn_out_c @ Wo[rows_c].  The host sums the 4 partials per batch
and adds bo.

Per-core kernel structure (PE kept continuously busy to hold the HAM
clock gate at K=8/8):
  - QKV: x^T and W in bf16 (halved DMA, FWL weight loads).  qT/kT
    [128, 2, 2048] fp32 with heads-on-partitions (pair member a at
    partition a*64+dv); bias folded into the PSUM->SBUF evacuation
    (DVE tensor_scalar add, with the 1/8 softmax scale folded into qT),
    v bias via a K=1 rank-1 matmul accumulated into the same PSUM.
  - Attention: per (sqc, hp, kb) one 128-key block: two K=64 score
    matmuls (one per pair member) into separate 1-bank PSUM tiles,
    software-pipelined one step ahead of the attnV matmuls so the PE
    never waits on exp.  exp is split between the Scalar engine
    (member a=0, table exp) and the Vector engine (member a=1,
    Schraudolph: ex = bitcast_bf16(round_i16(x*2^7/ln2 + B))) to keep
    both under the PE's per-block budget, two blocks ahead of the attnV
    matmuls so the PE never waits.  v carries an extra ones column so
    attnV also accumulates the softmax denominator (PSUM row 64).
  - Denominators DMA straight from PSUM row 64 to DRAM, get spread
    over 64 partitions for a parallel reciprocal, and broadcast back --
    all overlapped with the next chunk's attention.
  - Output projection runs one sq-chunk behind attention with K=128
    (head pairs stacked on partitions via partition-shifted evacuation
    copies), N=512 bf16 matmuls against Wo, interleaved into fixed slots
    of the following chunk so no engine sees a burst; normalization on
    GPSIMD; fp16 output partials summed on the host in fp32.
"""

import numpy as np
import ml_dtypes

S = 2048
D = 1024
H = 16
DEPTH = 64
NCORES = 8
GH = 4              # heads per core
GD = GH * DEPTH     # 256 projection outputs per core
KC = 8              # contraction chunks of 128 (K = 1024)

SCHR_A = 184.6650390625          # 2**7 / ln 2
SCHR_B = 16250.0                 # 127 * 2**7 - C

_state = {}


def _build():
    import concourse.mybir as mybir
    import concourse.tile as tile
    from concourse import bacc
    from concourse.bass import ts

    fp32 = mybir.dt.float32
    fp32r = mybir.dt.float32r
    bf16 = mybir.dt.bfloat16
    i16 = mybir.dt.int16
    fp16 = mybir.dt.float16
    Exp = mybir.ActivationFunctionType.Exp
    Copy = mybir.ActivationFunctionType.Copy
    Alu = mybir.AluOpType

    nc = bacc.Bacc("TRN2", target_bir_lowering=False, debug=False)
    xT = nc.dram_tensor("xT", [D, S], bf16, kind="ExternalInput")
    wq = nc.dram_tensor("wq", [D, GD], bf16, kind="ExternalInput")
    wk = nc.dram_tensor("wk", [D, GD], bf16, kind="ExternalInput")
    wv = nc.dram_tensor("wv", [D, GD], bf16, kind="ExternalInput")
    wo = nc.dram_tensor("wo", [GD, D], bf16, kind="ExternalInput")
    bqk = nc.dram_tensor("bqk", [128, 4], fp32, kind="ExternalInput")
    bv_d = nc.dram_tensor("bv", [1, GD], bf16, kind="ExternalInput")
    out = nc.dram_tensor("out", [S, D], fp16, kind="ExternalOutput")
    # denominators, flat [sqc, hp, a, s] (a = pair member)
    raw_dram = nc.dram_tensor("denom_raw", [4 * 4 * 512], fp32)
    rec_dram = nc.dram_tensor("denom_rec", [4 * 4 * 512], bf16)

    xT_view = xT[:].rearrange("(c p) s -> p c s", p=128)

    with tile.TileContext(nc) as tc:
        with tc.tile_pool(name="singles", bufs=1) as singles:
            qT = singles.tile([128, 2, S], fp32r)     # [a*64+dv, hp, sq]
            kT = singles.tile([128, 2, S], fp32r)
            v_sb = singles.tile([128, 16, GH, DEPTH + 1], bf16)   # v_aug
            outTs = singles.tile([128, 2, S], bf16)   # [a*64+dv, hp, sq]
            wo_sb = singles.tile([128, 2, D], bf16)   # [a*64+dv, hp, n]
            bqk_sb = singles.tile([128, 4], fp32)     # [p, (bq hp0, bq hp1, bk hp0, bk hp1)]
            bv_sb = singles.tile([1, GD], bf16)
            ones_sb = singles.tile([1, 128], bf16)
            dummy = singles.tile([1, 8], fp32)
            nc.vector.memset(v_sb[:, :, :, DEPTH : DEPTH + 1], 1.0)
            nc.vector.memset(ones_sb[:], 1.0)
            nc.vector.memset(dummy[:], 0.0)
            # force the exp table set to load long before attention starts
            nc.scalar.activation(dummy[:], dummy[:], Exp)

            # ---------- phase 1: QKV projections ----------
            with (
                tc.tile_pool(name="wpool", bufs=1) as wpool,
                tc.tile_pool(name="xpool", bufs=2) as xpool,
                tc.tile_pool(name="ps1", bufs=4, space="PSUM") as ps1,
            ):
                wq_sb = wpool.tile([128, KC, GD], bf16)
                wk_sb = wpool.tile([128, KC, GD], bf16)
                wv_sb = wpool.tile([128, KC, GD], bf16)
                xc0 = wpool.tile([128, KC, 512], bf16)
                # startup order: first-needed first
                nc.sync.dma_start(wq_sb[:, 0, :], wq[0:128, :])
                nc.sync.dma_start(xc0[:, 0, :], xT_view[:, 0, 0:512])
                nc.sync.dma_start(
                    wq_sb[:, 1:KC, :],
                    wq[128:D, :].rearrange("(c p) d -> p c d", p=128),
                )
                for kc in range(1, KC):
                    nc.sync.dma_start(xc0[:, kc, :], xT_view[:, kc, 0:512])
                nc.sync.dma_start(
                    wk_sb[:], wk[:].rearrange("(c p) d -> p c d", p=128)
                )
                nc.sync.dma_start(
                    wv_sb[:], wv[:].rearrange("(c p) d -> p c d", p=128)
                )
                nc.sync.dma_start(bqk_sb[:], bqk[:])
                nc.sync.dma_start(bv_sb[:], bv_d[:])
                nc.sync.dma_start(
                    wo_sb[:], wo[:].rearrange("(hp p) n -> p hp n", p=128)
                )

                for sc in range(4):  # s-chunks of 512
                    if sc == 0:
                        xc = xc0
                    else:
                        xc = xpool.tile([128, KC, 512], bf16, tag="xc", name="xc")
                        nc.sync.dma_start(xc[:], xT_view[:, :, ts(sc, 512)])
                    for hp in range(2):
                        for wi, (w_sb, dst) in enumerate(
                            ((wq_sb, qT), (wk_sb, kT))
                        ):
                            ps = ps1.tile([128, 512], fp32, tag="pq", name="psq")
                            for kc in range(KC):
                                nc.tensor.matmul(
                                    ps[:],
                                    w_sb[:, kc, ts(hp, 128)],
                                    xc[:, kc, :],
                                    start=(kc == 0),
                                    stop=(kc == KC - 1),
                                )
                            # evac with fused bias (and 1/8 scale for q)
                            if wi == 0:
                                nc.vector.tensor_scalar(
                                    dst[:, hp, ts(sc, 512)],
                                    ps[:],
                                    bqk_sb[:, hp : hp + 1],
                                    0.125,
                                    Alu.add,
                                    Alu.mult,
                                )
                            else:
                                nc.vector.tensor_scalar(
                                    dst[:, hp, ts(sc, 512)],
                                    ps[:],
                                    bqk_sb[:, 2 + hp : 3 + hp],
                                    None,
                                    Alu.add,
                                )
                    for mm in range(4):  # s-blocks of 128 inside the chunk
                        ps = ps1.tile([128, GD], fp32, tag="pv", name="psv")
                        nc.tensor.matmul(
                            ps[:], ones_sb[:], bv_sb[:], start=True, stop=False
                        )
                        for kc in range(KC):
                            nc.tensor.matmul(
                                ps[:],
                                xc[:, kc, ts(mm, 128)],
                                wv_sb[:, kc, :],
                                start=False,
                                stop=(kc == KC - 1),
                            )
                        nc.vector.tensor_copy(
                            v_sb[:, sc * 4 + mm, :, 0:DEPTH],
                            ps[:].rearrange("p (h d) -> p h d", h=GH),
                        )

            # ---------- phase 2: attention + interleaved projection ----------
            with (
                tc.tile_pool(name="expp", bufs=3) as expp,
                tc.tile_pool(name="onp", bufs=2) as onp,
                tc.tile_pool(name="rbp", bufs=2) as rbp,
                tc.tile_pool(name="outp", bufs=3) as outp,
                tc.tile_pool(name="pss", bufs=6, space="PSUM") as pss,
                tc.tile_pool(name="pso", bufs=2, space="PSUM") as pso,
            ):
                NT = 128  # (sqc, hp, kb) flat iterations
                SKEW = 2  # scores run this many blocks ahead of attnV
                pend = {}
                oX_by_hp = {}
                rb_by_sqc = {}
                rt_by = {}
                outN_by = {}

                def head_block(t):
                    return t // 32, (t // 16) % 2, t % 16  # sqc, hp, kb

                def emit_chain(sqc, hp):
                    # one hp's denominators: spread over 32 partitions,
                    # reciprocal, write back bf16, broadcast into rb
                    base = sqc * 2048 + hp * 1024
                    rr = rbp.tile([32, 32], fp32, tag="rr", name="rr")
                    nc.sync.dma_start(
                        rr[:],
                        raw_dram[base : base + 1024].rearrange(
                            "(p x) -> p x", p=32
                        ),
                    )
                    rr2 = rbp.tile([32, 32], bf16, tag="rr2", name="rr2")
                    with nc.allow_low_precision(reason="bf16 denom recip"):
                        nc.vector.reciprocal(rr2[:], rr[:])
                    nc.sync.dma_start(
                        rec_dram[base : base + 1024].rearrange(
                            "(p x) -> p x", p=32
                        ),
                        rr2[:],
                    )
                    if hp == 0:
                        rb_by_sqc[sqc] = rbp.tile(
                            [128, 2, 512], bf16, tag="rb", name="rb"
                        )
                    rb = rb_by_sqc[sqc]
                    for a in range(2):
                        rec_v = rec_dram[
                            base + a * 512 : base + (a + 1) * 512
                        ].rearrange("(p h s) -> p h s", p=1, h=1)
                        nc.sync.dma_start(
                            rb[ts(a, 64), ts(hp, 1), :],
                            rec_v.to_broadcast([64, 1, 512]),
                        )

                def emit_norm(sqc, mm, eng=None):
                    # normalize into a fresh tile: in-place on outTs would
                    # push it past tile's writer limit and coarsen deps
                    if mm == 0:
                        outN_by[sqc] = onp.tile(
                            [128, 2, 512], bf16, tag="on", name="on"
                        )
                    m = sqc * 4 + mm
                    eng = eng or nc.gpsimd
                    with nc.allow_low_precision(reason="bf16 softmax norm"):
                        eng.tensor_mul(
                            outN_by[sqc][:, :, ts(mm, 128)],
                            outTs[:, :, ts(m, 128)],
                            rb_by_sqc[sqc][:, :, ts(mm, 128)],
                        )

                def emit_pp(sqc, mm):
                    m = sqc * 4 + mm
                    for nn in range(2):
                        # borrow a transient scores slot (the "o"
                        # accumulators live a whole hp block and would
                        # deadlock the PE queue)
                        pp = pss.tile([128, 512], fp32, tag="s", name="pp")
                        for hp in range(2):
                            nc.tensor.matmul(
                                pp[:],
                                outN_by[sqc][:, hp, ts(mm, 128)],
                                wo_sb[:, hp, ts(nn, 512)],
                                start=(hp == 0),
                                stop=(hp == 1),
                            )
                        ot = outp.tile([128, 512], fp16, tag="ot", name="ot")
                        if nn == 0:
                            nc.scalar.activation(ot[:], pp[:], Copy)
                        else:
                            nc.vector.tensor_copy(ot[:], pp[:])
                        nc.sync.dma_start(out[ts(m, 128), ts(nn, 512)], ot[:])

                for t in range(NT + SKEW):
                    if t < NT:
                        sqc, hp, kb = head_block(t)
                        if kb == 0:
                            oX_by_hp[t // 16] = [
                                pso.tile([65, 512], fp32, tag="o", name=f"o{a}")
                                for a in range(2)
                            ]
                        # scores for block t (SKEW ahead of attnV)
                        sps = [
                            pss.tile([128, 512], fp32, tag="s", name=f"s{a}")
                            for a in range(2)
                        ]
                        for a in range(2):
                            nc.tensor.matmul(
                                sps[a][:],
                                kT[ts(a, 64), hp, ts(kb, 128)],
                                qT[ts(a, 64), hp, ts(sqc, 512)],
                                start=True,
                                stop=True,
                            )
                        ex = expp.tile([128, 2, 512], bf16, tag="e", bufs=4, name="ex")
                        nc.scalar.activation(ex[:, 0, :], sps[0][:], Exp)
                        nc.vector.tensor_scalar(
                            ex[:, 1, :].bitcast(i16),
                            sps[1][:],
                            SCHR_A,
                            SCHR_B,
                            Alu.mult,
                            Alu.add,
                        )
                        pend[t] = ex
                        # denominator chains and the previous chunk's
                        # projection, spread across this chunk's iterations
                        # so no engine sees a burst
                        r = t % 32
                        if r == 24:
                            emit_chain(sqc, 0)
                        elif r == 4 and sqc > 0:
                            emit_chain(sqc - 1, 1)
                        if 0 < sqc < 3 and r in (10, 14, 18, 22, 26):
                            j = (r - 10) // 4
                            if j < 4:
                                emit_norm(sqc - 1, j)
                            if j > 0:
                                emit_pp(sqc - 1, j - 1)
                        if 0 < sqc < 3 and r == 30:
                            emit_pp(sqc - 1, 3)

                    tp = t - SKEW
                    if tp >= 0:
                        exp_t = pend.pop(tp)
                        psqc, php, pkb = head_block(tp)
                        oXp = oX_by_hp[tp // 16]
                        for a in range(2):
                            nc.tensor.matmul(
                                oXp[a][:],
                                v_sb[:, pkb, 2 * php + a, :],
                                exp_t[:, a, :],
                                start=(pkb == 0),
                                stop=(pkb == 15),
                            )
                        if pkb == 15:
                            # evacuate: stack pair members on partitions;
                            # denominator rows go through an SBUF scratch on
                            # ScalarE (DMA cannot read PSUM directly)
                            rt = rbp.tile(
                                [65, 2, 512], fp32, tag="rt", name="rt"
                            )
                            rt_by[(psqc, php)] = rt
                            for a in range(2):
                                nc.vector.tensor_copy(
                                    outTs[ts(a, 64), php, ts(psqc, 512)],
                                    oXp[a][0:64, :],
                                )
                                nc.scalar.activation(
                                    rt[64:65, a, :], oXp[a][64:65, :], Copy
                                )
                                ofs = psqc * 2048 + php * 1024 + a * 512
                                nc.sync.dma_start(
                                    raw_dram[ofs : ofs + 512].rearrange(
                                        "(p s) -> p s", p=1
                                    ),
                                    rt[64:65, a, :],
                                )

                # tail: normalize chunk 2 (ready), kick the last chunk's
                # hp1 chain, fill its DMA latency with chunk 2's projection
                # (emitting the chain after the norms keeps proj(2) off the
                # chain's cumulative DMA-semaphore thresholds), then chunk 3
                for mm in range(4):
                    emit_norm(2, mm, eng=nc.vector)
                emit_chain(3, 1)
                for mm in range(4):
                    emit_pp(2, mm)
                for mm in range(4):
                    emit_norm(3, mm, eng=nc.vector)
                    emit_pp(3, mm)

    nc.compile()
    return nc


def _get_nc():
    if "nc" not in _state:
        _state["nc"] = _build()
    return _state["nc"]


def _prep_core_inputs(inputs, Wq, bq, Wk, bk, Wv, bv, Wo, bo):
    bf = ml_dtypes.bfloat16
    in_maps = []
    for c in range(NCORES):
        b, g = divmod(c, 4)
        cols = slice(g * GD, (g + 1) * GD)
        bqk = np.zeros((128, 4), np.float32)
        bqk[:, 0:2] = bq[cols].reshape(2, 128).T
        bqk[:, 2:4] = bk[cols].reshape(2, 128).T
        m = {
            "xT": np.ascontiguousarray(inputs[b].T).astype(bf),
            "wq": np.ascontiguousarray(Wq[:, cols]).astype(bf),
            "wk": np.ascontiguousarray(Wk[:, cols]).astype(bf),
            "wv": np.ascontiguousarray(Wv[:, cols]).astype(bf),
            "wo": np.ascontiguousarray(Wo[cols, :]).astype(bf),
            "bqk": bqk,
            "bv": bv[cols].reshape(1, GD).astype(bf),
        }
        in_maps.append(m)
    return in_maps


def run(inputs, Wq, bq, Wk, bk, Wv, bv, Wo, bo, trace=False):
    from concourse.bass_utils import run_bass_kernel_spmd

    nc = _get_nc()
    in_maps = _prep_core_inputs(inputs, Wq, bq, Wk, bk, Wv, bv, Wo, bo)
    res = run_bass_kernel_spmd(
        nc, in_maps, core_ids=list(range(NCORES)), trace=trace
    )
    out = np.zeros((2, S, D), np.float32)
    for c in range(NCORES):
        out[c // 4] += res.results[c]["out"].astype(np.float32)
    out += np.asarray(bo, np.float32)
    return out, res


def kernel(inputs, Wq, bq, Wk, bk, Wv, bv, Wo, bo):
    out, _ = run(
        np.asarray(inputs, np.float32),
        np.asarray(Wq, np.float32), np.asarray(bq, np.float32),
        np.asarray(Wk, np.float32), np.asarray(bk, np.float32),
        np.asarray(Wv, np.float32), np.asarray(bv, np.float32),
        np.asarray(Wo, np.float32), np.asarray(bo, np.float32),
    )
    return out


# revision 23
# speedup vs baseline: 1.1660x; 1.1660x over previous
"""Multi-head attention (B=2, S=2048, D=1024, H=16) on 8 NeuronCores.

Sharding: core c -> batch b = c//4, head group g = c%4 (4 heads each).
Each core computes q/k/v projections for its head group, full softmax
attention for its 4 heads, and a partial output projection
out_c = attn_out_c @ Wo[rows_c].  The host sums the 4 partials per batch
and adds bo.

Per-core kernel structure (PE kept continuously busy to hold the HAM
clock gate at K=8/8):
  - QKV: x^T and W in bf16 (halved DMA, FWL weight loads).  qT/kT
    [128, 2, 2048] fp32 with heads-on-partitions (pair member a at
    partition a*64+dv); bias folded into the PSUM->SBUF evacuation
    (DVE tensor_scalar add, with the 1/8 softmax scale folded into qT),
    v bias via a K=1 rank-1 matmul accumulated into the same PSUM.
  - Attention: per (sqc, hp, kb) one 128-key block: two K=64 score
    matmuls (one per pair member) into separate 1-bank PSUM tiles,
    software-pipelined one step ahead of the attnV matmuls so the PE
    never waits on exp.  exp is split between the Scalar engine
    (member a=0, table exp) and the Vector engine (member a=1,
    Schraudolph: ex = bitcast_bf16(round_i16(x*2^7/ln2 + B))) to keep
    both under the PE's per-block budget, two blocks ahead of the attnV
    matmuls so the PE never waits.  v carries an extra ones column so
    attnV also accumulates the softmax denominator (PSUM row 64).
  - Denominators DMA straight from PSUM row 64 to DRAM, get spread
    over 64 partitions for a parallel reciprocal, and broadcast back --
    all overlapped with the next chunk's attention.
  - Output projection runs one sq-chunk behind attention with K=128
    (head pairs stacked on partitions via partition-shifted evacuation
    copies), N=512 bf16 matmuls against Wo, interleaved into fixed slots
    of the following chunk so no engine sees a burst; normalization on
    GPSIMD; fp16 output partials summed on the host in fp32.
"""

import numpy as np
import ml_dtypes

S = 2048
D = 1024
H = 16
DEPTH = 64
NCORES = 8
GH = 4              # heads per core
GD = GH * DEPTH     # 256 projection outputs per core
KC = 8              # contraction chunks of 128 (K = 1024)

SCHR_A = 184.6650390625          # 2**7 / ln 2
SCHR_B = 16250.0                 # 127 * 2**7 - C

_state = {}


def _build():
    import concourse.mybir as mybir
    import concourse.tile as tile
    from concourse import bacc
    from concourse.bass import ts

    fp32 = mybir.dt.float32
    fp32r = mybir.dt.float32r
    bf16 = mybir.dt.bfloat16
    i16 = mybir.dt.int16
    fp16 = mybir.dt.float16
    Exp = mybir.ActivationFunctionType.Exp
    Copy = mybir.ActivationFunctionType.Copy
    Alu = mybir.AluOpType

    nc = bacc.Bacc("TRN2", target_bir_lowering=False, debug=False)
    xT = nc.dram_tensor("xT", [D, S], bf16, kind="ExternalInput")
    wq = nc.dram_tensor("wq", [D, GD], bf16, kind="ExternalInput")
    wk = nc.dram_tensor("wk", [D, GD], bf16, kind="ExternalInput")
    wv = nc.dram_tensor("wv", [D, GD], bf16, kind="ExternalInput")
    wo = nc.dram_tensor("wo", [GD, D], bf16, kind="ExternalInput")
    bqk = nc.dram_tensor("bqk", [128, 4], fp32, kind="ExternalInput")
    bv_d = nc.dram_tensor("bv", [1, GD], bf16, kind="ExternalInput")
    out = nc.dram_tensor("out", [S, D], fp16, kind="ExternalOutput")
    # denominators, flat [sqc, hp, a, s] (a = pair member)
    raw_dram = nc.dram_tensor("denom_raw", [4 * 4 * 512], fp32)
    rec_dram = nc.dram_tensor("denom_rec", [4 * 4 * 512], bf16)

    xT_view = xT[:].rearrange("(c p) s -> p c s", p=128)

    with tile.TileContext(nc) as tc:
        with tc.tile_pool(name="singles", bufs=1) as singles:
            qT = singles.tile([128, 2, S], fp32r)     # [a*64+dv, hp, sq]
            kT = singles.tile([128, 2, S], fp32r)
            v_sb = singles.tile([128, 16, GH, DEPTH + 1], bf16)   # v_aug
            outTs = singles.tile([128, 2, S], bf16)   # [a*64+dv, hp, sq]
            wo_sb = singles.tile([128, 2, D], bf16)   # [a*64+dv, hp, n]
            bqk_sb = singles.tile([128, 4], fp32)     # [p, (bq hp0, bq hp1, bk hp0, bk hp1)]
            bv_sb = singles.tile([1, GD], bf16)
            ones_sb = singles.tile([1, 128], bf16)
            dummy = singles.tile([1, 8], fp32)
            nc.vector.memset(v_sb[:, :, :, DEPTH : DEPTH + 1], 1.0)
            nc.vector.memset(ones_sb[:], 1.0)
            nc.vector.memset(dummy[:], 0.0)
            # force the exp table set to load long before attention starts
            nc.scalar.activation(dummy[:], dummy[:], Exp)

            # ---------- phase 1: QKV projections ----------
            with (
                tc.tile_pool(name="wpool", bufs=1) as wpool,
                tc.tile_pool(name="xpool", bufs=2) as xpool,
                tc.tile_pool(name="ps1", bufs=4, space="PSUM") as ps1,
            ):
                wq_sb = wpool.tile([128, KC, GD], bf16)
                wk_sb = wpool.tile([128, KC, GD], bf16)
                wv_sb = wpool.tile([128, KC, GD], bf16)
                xc0 = wpool.tile([128, KC, 512], bf16)
                # startup order: first-needed first
                nc.sync.dma_start(wq_sb[:, 0, :], wq[0:128, :])
                nc.sync.dma_start(xc0[:, 0, :], xT_view[:, 0, 0:512])
                nc.sync.dma_start(
                    wq_sb[:, 1:KC, :],
                    wq[128:D, :].rearrange("(c p) d -> p c d", p=128),
                )
                for kc in range(1, KC):
                    nc.sync.dma_start(xc0[:, kc, :], xT_view[:, kc, 0:512])
                nc.sync.dma_start(
                    wk_sb[:], wk[:].rearrange("(c p) d -> p c d", p=128)
                )
                nc.sync.dma_start(
                    wv_sb[:], wv[:].rearrange("(c p) d -> p c d", p=128)
                )
                nc.sync.dma_start(bqk_sb[:], bqk[:])
                nc.sync.dma_start(bv_sb[:], bv_d[:])
                nc.sync.dma_start(
                    wo_sb[:], wo[:].rearrange("(hp p) n -> p hp n", p=128)
                )

                for sc in range(4):  # s-chunks of 512
                    if sc == 0:
                        xc = xc0
                    else:
                        xc = xpool.tile([128, KC, 512], bf16, tag="xc", name="xc")
                        nc.sync.dma_start(xc[:], xT_view[:, :, ts(sc, 512)])
                    for hp in range(2):
                        for wi, (w_sb, dst) in enumerate(
                            ((wq_sb, qT), (wk_sb, kT))
                        ):
                            ps = ps1.tile([128, 512], fp32, tag="pq", name="psq")
                            for kc in range(KC):
                                nc.tensor.matmul(
                                    ps[:],
                                    w_sb[:, kc, ts(hp, 128)],
                                    xc[:, kc, :],
                                    start=(kc == 0),
                                    stop=(kc == KC - 1),
                                )
                            # evac with fused bias (and 1/8 scale for q)
                            if wi == 0:
                                nc.vector.tensor_scalar(
                                    dst[:, hp, ts(sc, 512)],
                                    ps[:],
                                    bqk_sb[:, hp : hp + 1],
                                    0.125,
                                    Alu.add,
                                    Alu.mult,
                                )
                            else:
                                nc.vector.tensor_scalar(
                                    dst[:, hp, ts(sc, 512)],
                                    ps[:],
                                    bqk_sb[:, 2 + hp : 3 + hp],
                                    None,
                                    Alu.add,
                                )
                    for mm in range(4):  # s-blocks of 128 inside the chunk
                        ps = ps1.tile([128, GD], fp32, tag="pv", name="psv")
                        nc.tensor.matmul(
                            ps[:], ones_sb[:], bv_sb[:], start=True, stop=False
                        )
                        for kc in range(KC):
                            nc.tensor.matmul(
                                ps[:],
                                xc[:, kc, ts(mm, 128)],
                                wv_sb[:, kc, :],
                                start=False,
                                stop=(kc == KC - 1),
                            )
                        nc.vector.tensor_copy(
                            v_sb[:, sc * 4 + mm, :, 0:DEPTH],
                            ps[:].rearrange("p (h d) -> p h d", h=GH),
                        )

            # ---------- phase 2: attention + interleaved projection ----------
            with (
                tc.tile_pool(name="expp", bufs=3) as expp,
                tc.tile_pool(name="onp", bufs=2) as onp,
                tc.tile_pool(name="rbp", bufs=2) as rbp,
                tc.tile_pool(name="outp", bufs=3) as outp,
                tc.tile_pool(name="pss", bufs=6, space="PSUM") as pss,
                tc.tile_pool(name="pso", bufs=2, space="PSUM") as pso,
            ):
                NT = 128  # (sqc, hp, kb) flat iterations
                SKEW = 2  # scores run this many blocks ahead of attnV
                pend = {}
                oX_by_hp = {}
                rb_by_sqc = {}
                rt_by = {}
                outN_by = {}

                def head_block(t):
                    return t // 32, (t // 16) % 2, t % 16  # sqc, hp, kb

                def emit_chain(sqc, hp):
                    # one hp's denominators: spread over 32 partitions,
                    # reciprocal, write back bf16, broadcast into rb
                    base = sqc * 2048 + hp * 1024
                    rr = rbp.tile([32, 32], fp32, tag="rr", name="rr")
                    nc.sync.dma_start(
                        rr[:],
                        raw_dram[base : base + 1024].rearrange(
                            "(p x) -> p x", p=32
                        ),
                    )
                    rr2 = rbp.tile([32, 32], bf16, tag="rr2", name="rr2")
                    with nc.allow_low_precision(reason="bf16 denom recip"):
                        nc.vector.reciprocal(rr2[:], rr[:])
                    nc.sync.dma_start(
                        rec_dram[base : base + 1024].rearrange(
                            "(p x) -> p x", p=32
                        ),
                        rr2[:],
                    )
                    if hp == 0:
                        rb_by_sqc[sqc] = rbp.tile(
                            [128, 2, 512], bf16, tag="rb", name="rb"
                        )
                    rb = rb_by_sqc[sqc]
                    for a in range(2):
                        rec_v = rec_dram[
                            base + a * 512 : base + (a + 1) * 512
                        ].rearrange("(p h s) -> p h s", p=1, h=1)
                        nc.sync.dma_start(
                            rb[ts(a, 64), ts(hp, 1), :],
                            rec_v.to_broadcast([64, 1, 512]),
                        )

                def emit_norm(sqc, mm, eng=None):
                    # normalize into a fresh tile: in-place on outTs would
                    # push it past tile's writer limit and coarsen deps
                    if mm == 0:
                        outN_by[sqc] = onp.tile(
                            [128, 2, 512], bf16, tag="on", name="on"
                        )
                    m = sqc * 4 + mm
                    eng = eng or nc.gpsimd
                    with nc.allow_low_precision(reason="bf16 softmax norm"):
                        eng.tensor_mul(
                            outN_by[sqc][:, :, ts(mm, 128)],
                            outTs[:, :, ts(m, 128)],
                            rb_by_sqc[sqc][:, :, ts(mm, 128)],
                        )

                def emit_pp(sqc, mm):
                    m = sqc * 4 + mm
                    for nn in range(2):
                        # borrow a transient scores slot (the "o"
                        # accumulators live a whole hp block and would
                        # deadlock the PE queue)
                        pp = pss.tile([128, 512], fp32, tag="s", name="pp")
                        for hp in range(2):
                            nc.tensor.matmul(
                                pp[:],
                                outN_by[sqc][:, hp, ts(mm, 128)],
                                wo_sb[:, hp, ts(nn, 512)],
                                start=(hp == 0),
                                stop=(hp == 1),
                            )
                        ot = outp.tile([128, 512], fp16, tag="ot", name="ot")
                        if nn == 0:
                            nc.scalar.activation(ot[:], pp[:], Copy)
                        else:
                            nc.vector.tensor_copy(ot[:], pp[:])
                        nc.sync.dma_start(out[ts(m, 128), ts(nn, 512)], ot[:])

                for t in range(NT + SKEW):
                    if t < NT:
                        sqc, hp, kb = head_block(t)
                        if kb == 0:
                            oX_by_hp[t // 16] = [
                                pso.tile([65, 512], fp32, tag="o", name=f"o{a}")
                                for a in range(2)
                            ]
                        # scores for block t (SKEW ahead of attnV)
                        sps = [
                            pss.tile([128, 512], fp32, tag="s", name=f"s{a}")
                            for a in range(2)
                        ]
                        for a in range(2):
                            nc.tensor.matmul(
                                sps[a][:],
                                kT[ts(a, 64), hp, ts(kb, 128)],
                                qT[ts(a, 64), hp, ts(sqc, 512)],
                                start=True,
                                stop=True,
                            )
                        ex = expp.tile([128, 2, 512], bf16, tag="e", bufs=4, name="ex")
                        nc.scalar.activation(ex[:, 0, :], sps[0][:], Exp)
                        nc.vector.tensor_scalar(
                            ex[:, 1, :].bitcast(i16),
                            sps[1][:],
                            SCHR_A,
                            SCHR_B,
                            Alu.mult,
                            Alu.add,
                        )
                        pend[t] = ex
                        # denominator chains and the previous chunk's
                        # projection, spread across this chunk's iterations
                        # so no engine sees a burst
                        r = t % 32
                        if r == 24:
                            emit_chain(sqc, 0)
                        elif r == 4 and sqc > 0:
                            emit_chain(sqc - 1, 1)
                        if 0 < sqc < 3 and r in (10, 14, 18, 22, 26):
                            j = (r - 10) // 4
                            if j < 4:
                                emit_norm(sqc - 1, j)
                            if j > 0:
                                emit_pp(sqc - 1, j - 1)
                        if 0 < sqc < 3 and r == 30:
                            emit_pp(sqc - 1, 3)

                    tp = t - SKEW
                    if tp >= 0:
                        exp_t = pend.pop(tp)
                        psqc, php, pkb = head_block(tp)
                        oXp = oX_by_hp[tp // 16]
                        for a in range(2):
                            nc.tensor.matmul(
                                oXp[a][:],
                                v_sb[:, pkb, 2 * php + a, :],
                                exp_t[:, a, :],
                                start=(pkb == 0),
                                stop=(pkb == 15),
                            )
                        if pkb == 15:
                            deferred = []
                            if psqc == 3 and php == 1:
                                # project chunk 2 NOW: before this block's
                                # evac/raw emissions so its semaphore
                                # thresholds exclude the last chain's DMAs
                                # (they are cumulative per rotating sem),
                                # and so its matmuls fill the PE during the
                                # chain's DMA latency.  Output DMAs are
                                # deferred behind the raws so the chain
                                # isn't queued after 1MB of output traffic.
                                for mm in range(4):
                                    emit_norm(2, mm, eng=nc.vector)
                                    for nn in range(2):
                                        pp = pss.tile(
                                            [128, 512], fp32,
                                            tag="s", name="pp",
                                        )
                                        for h2 in range(2):
                                            nc.tensor.matmul(
                                                pp[:],
                                                outN_by[2][:, h2, ts(mm, 128)],
                                                wo_sb[:, h2, ts(nn, 512)],
                                                start=(h2 == 0),
                                                stop=(h2 == 1),
                                            )
                                        ot = outp.tile(
                                            [128, 512], fp16,
                                            tag="ot2", bufs=8, name="ot2",
                                        )
                                        if nn == 0:
                                            nc.scalar.activation(
                                                ot[:], pp[:], Copy
                                            )
                                        else:
                                            nc.vector.tensor_copy(
                                                ot[:], pp[:]
                                            )
                                        deferred.append((8 + mm, nn, ot))
                            # evacuate: stack pair members on partitions;
                            # denominator rows go through an SBUF scratch on
                            # ScalarE (DMA cannot read PSUM directly)
                            rt = rbp.tile(
                                [65, 2, 512], fp32, tag="rt", name="rt"
                            )
                            rt_by[(psqc, php)] = rt
                            for a in range(2):
                                nc.vector.tensor_copy(
                                    outTs[ts(a, 64), php, ts(psqc, 512)],
                                    oXp[a][0:64, :],
                                )
                                nc.scalar.activation(
                                    rt[64:65, a, :], oXp[a][64:65, :], Copy
                                )
                                ofs = psqc * 2048 + php * 1024 + a * 512
                                nc.sync.dma_start(
                                    raw_dram[ofs : ofs + 512].rearrange(
                                        "(p s) -> p s", p=1
                                    ),
                                    rt[64:65, a, :],
                                )
                            for m, nn, ot in deferred:
                                nc.sync.dma_start(
                                    out[ts(m, 128), ts(nn, 512)], ot[:]
                                )

                # tail: the last chunk's hp1 chain, then its projection
                emit_chain(3, 1)
                for mm in range(4):
                    emit_norm(3, mm, eng=nc.vector)
                    emit_pp(3, mm)

    nc.compile()
    return nc


def _get_nc():
    if "nc" not in _state:
        _state["nc"] = _build()
    return _state["nc"]


def _prep_core_inputs(inputs, Wq, bq, Wk, bk, Wv, bv, Wo, bo):
    bf = ml_dtypes.bfloat16
    in_maps = []
    for c in range(NCORES):
        b, g = divmod(c, 4)
        cols = slice(g * GD, (g + 1) * GD)
        bqk = np.zeros((128, 4), np.float32)
        bqk[:, 0:2] = bq[cols].reshape(2, 128).T
        bqk[:, 2:4] = bk[cols].reshape(2, 128).T
        m = {
            "xT": np.ascontiguousarray(inputs[b].T).astype(bf),
            "wq": np.ascontiguousarray(Wq[:, cols]).astype(bf),
            "wk": np.ascontiguousarray(Wk[:, cols]).astype(bf),
            "wv": np.ascontiguousarray(Wv[:, cols]).astype(bf),
            "wo": np.ascontiguousarray(Wo[cols, :]).astype(bf),
            "bqk": bqk,
            "bv": bv[cols].reshape(1, GD).astype(bf),
        }
        in_maps.append(m)
    return in_maps


def run(inputs, Wq, bq, Wk, bk, Wv, bv, Wo, bo, trace=False):
    from concourse.bass_utils import run_bass_kernel_spmd

    nc = _get_nc()
    in_maps = _prep_core_inputs(inputs, Wq, bq, Wk, bk, Wv, bv, Wo, bo)
    res = run_bass_kernel_spmd(
        nc, in_maps, core_ids=list(range(NCORES)), trace=trace
    )
    out = np.zeros((2, S, D), np.float32)
    for c in range(NCORES):
        out[c // 4] += res.results[c]["out"].astype(np.float32)
    out += np.asarray(bo, np.float32)
    return out, res


def kernel(inputs, Wq, bq, Wk, bk, Wv, bv, Wo, bo):
    out, _ = run(
        np.asarray(inputs, np.float32),
        np.asarray(Wq, np.float32), np.asarray(bq, np.float32),
        np.asarray(Wk, np.float32), np.asarray(bk, np.float32),
        np.asarray(Wv, np.float32), np.asarray(bv, np.float32),
        np.asarray(Wo, np.float32), np.asarray(bo, np.float32),
    )
    return out


# revision 24
# speedup vs baseline: 1.1780x; 1.0103x over previous
"""Multi-head attention (B=2, S=2048, D=1024, H=16) on 8 NeuronCores.

Sharding: core c -> batch b = c//4, head group g = c%4 (4 heads each).
Each core computes q/k/v projections for its head group, full softmax
attention for its 4 heads, and a partial output projection
out_c = attn_out_c @ Wo[rows_c].  The host sums the 4 partials per batch
and adds bo.

Per-core kernel structure (PE kept continuously busy to hold the HAM
clock gate at K=8/8):
  - QKV: x^T and W in bf16 (halved DMA, FWL weight loads).  qT/kT
    [128, 2, 2048] fp32 with heads-on-partitions (pair member a at
    partition a*64+dv); bias folded into the PSUM->SBUF evacuation
    (DVE tensor_scalar add, with the 1/8 softmax scale folded into qT),
    v bias via a K=1 rank-1 matmul accumulated into the same PSUM.
  - Attention: per (sqc, hp, kb) one 128-key block: two K=64 score
    matmuls (one per pair member) into separate 1-bank PSUM tiles,
    software-pipelined one step ahead of the attnV matmuls so the PE
    never waits on exp.  exp is split between the Scalar engine
    (member a=0, table exp) and the Vector engine (member a=1,
    Schraudolph: ex = bitcast_bf16(round_i16(x*2^7/ln2 + B))) to keep
    both under the PE's per-block budget, two blocks ahead of the attnV
    matmuls so the PE never waits.  v carries an extra ones column so
    attnV also accumulates the softmax denominator (PSUM row 64).
  - Denominators DMA straight from PSUM row 64 to DRAM, get spread
    over 64 partitions for a parallel reciprocal, and broadcast back --
    all overlapped with the next chunk's attention.
  - Output projection runs one sq-chunk behind attention with K=128
    (head pairs stacked on partitions via partition-shifted evacuation
    copies), N=512 bf16 matmuls against Wo, interleaved into fixed slots
    of the following chunk so no engine sees a burst; normalization on
    GPSIMD; fp16 output partials summed on the host in fp32.
"""

import numpy as np
import ml_dtypes

S = 2048
D = 1024
H = 16
DEPTH = 64
NCORES = 8
GH = 4              # heads per core
GD = GH * DEPTH     # 256 projection outputs per core
KC = 8              # contraction chunks of 128 (K = 1024)

SCHR_A = 184.6650390625          # 2**7 / ln 2
SCHR_B = 16250.0                 # 127 * 2**7 - C

_state = {}


def _build():
    import concourse.mybir as mybir
    import concourse.tile as tile
    from concourse import bacc
    from concourse.bass import ts

    fp32 = mybir.dt.float32
    fp32r = mybir.dt.float32r
    bf16 = mybir.dt.bfloat16
    i16 = mybir.dt.int16
    fp16 = mybir.dt.float16
    Exp = mybir.ActivationFunctionType.Exp
    Copy = mybir.ActivationFunctionType.Copy
    Alu = mybir.AluOpType

    nc = bacc.Bacc("TRN2", target_bir_lowering=False, debug=False)
    xT = nc.dram_tensor("xT", [D, S], bf16, kind="ExternalInput")
    wq = nc.dram_tensor("wq", [D, GD], bf16, kind="ExternalInput")
    wk = nc.dram_tensor("wk", [D, GD], bf16, kind="ExternalInput")
    wv = nc.dram_tensor("wv", [D, GD], bf16, kind="ExternalInput")
    wo = nc.dram_tensor("wo", [GD, D], bf16, kind="ExternalInput")
    bqk = nc.dram_tensor("bqk", [128, 4], fp32, kind="ExternalInput")
    bv_d = nc.dram_tensor("bv", [1, GD], bf16, kind="ExternalInput")
    out = nc.dram_tensor("out", [S, D], fp16, kind="ExternalOutput")
    # denominators, flat [sqc, hp, a, s] (a = pair member)
    raw_dram = nc.dram_tensor("denom_raw", [4 * 4 * 512], fp32)
    rec_dram = nc.dram_tensor("denom_rec", [4 * 4 * 512], bf16)

    xT_view = xT[:].rearrange("(c p) s -> p c s", p=128)

    with tile.TileContext(nc) as tc:
        with tc.tile_pool(name="singles", bufs=1) as singles:
            qT = singles.tile([128, 2, S], fp32r)     # [a*64+dv, hp, sq]
            kT = singles.tile([128, 2, S], fp32r)
            v_sb = singles.tile([128, 16, GH, DEPTH + 1], bf16)   # v_aug
            outTs = singles.tile([128, 2, S], bf16)   # [a*64+dv, hp, sq]
            wo_sb = singles.tile([128, 2, D], bf16)   # [a*64+dv, hp, n]
            bqk_sb = singles.tile([128, 4], fp32)     # [p, (bq hp0, bq hp1, bk hp0, bk hp1)]
            bv_sb = singles.tile([1, GD], bf16)
            ones_sb = singles.tile([1, 128], bf16)
            dummy = singles.tile([1, 8], fp32)
            nc.vector.memset(v_sb[:, :, :, DEPTH : DEPTH + 1], 1.0)
            nc.vector.memset(ones_sb[:], 1.0)
            nc.vector.memset(dummy[:], 0.0)
            # force the exp table set to load long before attention starts
            nc.scalar.activation(dummy[:], dummy[:], Exp)

            # ---------- phase 1: QKV projections ----------
            with (
                tc.tile_pool(name="wpool", bufs=1) as wpool,
                tc.tile_pool(name="xpool", bufs=2) as xpool,
                tc.tile_pool(name="ps1", bufs=4, space="PSUM") as ps1,
            ):
                wq_sb = wpool.tile([128, KC, GD], bf16)
                wk_sb = wpool.tile([128, KC, GD], bf16)
                wv_sb = wpool.tile([128, KC, GD], bf16)
                xc0 = wpool.tile([128, KC, 512], bf16)
                # startup order: first-needed first
                nc.sync.dma_start(wq_sb[:, 0, :], wq[0:128, :])
                nc.sync.dma_start(xc0[:, 0, :], xT_view[:, 0, 0:512])
                nc.sync.dma_start(
                    wq_sb[:, 1:KC, :],
                    wq[128:D, :].rearrange("(c p) d -> p c d", p=128),
                )
                for kc in range(1, KC):
                    nc.sync.dma_start(xc0[:, kc, :], xT_view[:, kc, 0:512])
                nc.sync.dma_start(
                    wk_sb[:], wk[:].rearrange("(c p) d -> p c d", p=128)
                )
                nc.sync.dma_start(
                    wv_sb[:], wv[:].rearrange("(c p) d -> p c d", p=128)
                )
                nc.sync.dma_start(bqk_sb[:], bqk[:])
                nc.sync.dma_start(bv_sb[:], bv_d[:])
                nc.sync.dma_start(
                    wo_sb[:], wo[:].rearrange("(hp p) n -> p hp n", p=128)
                )

                for sc in range(4):  # s-chunks of 512
                    if sc == 0:
                        xc = xc0
                    else:
                        xc = xpool.tile([128, KC, 512], bf16, tag="xc", name="xc")
                        nc.sync.dma_start(xc[:], xT_view[:, :, ts(sc, 512)])
                    for hp in range(2):
                        for wi, (w_sb, dst) in enumerate(
                            ((wq_sb, qT), (wk_sb, kT))
                        ):
                            ps = ps1.tile([128, 512], fp32, tag="pq", name="psq")
                            for kc in range(KC):
                                nc.tensor.matmul(
                                    ps[:],
                                    w_sb[:, kc, ts(hp, 128)],
                                    xc[:, kc, :],
                                    start=(kc == 0),
                                    stop=(kc == KC - 1),
                                )
                            # evac with fused bias (and 1/8 scale for q)
                            if wi == 0:
                                nc.vector.tensor_scalar(
                                    dst[:, hp, ts(sc, 512)],
                                    ps[:],
                                    bqk_sb[:, hp : hp + 1],
                                    0.125,
                                    Alu.add,
                                    Alu.mult,
                                )
                            else:
                                nc.vector.tensor_scalar(
                                    dst[:, hp, ts(sc, 512)],
                                    ps[:],
                                    bqk_sb[:, 2 + hp : 3 + hp],
                                    None,
                                    Alu.add,
                                )
                    for mm in range(4):  # s-blocks of 128 inside the chunk
                        ps = ps1.tile([128, GD], fp32, tag="pv", name="psv")
                        nc.tensor.matmul(
                            ps[:], ones_sb[:], bv_sb[:], start=True, stop=False
                        )
                        for kc in range(KC):
                            nc.tensor.matmul(
                                ps[:],
                                xc[:, kc, ts(mm, 128)],
                                wv_sb[:, kc, :],
                                start=False,
                                stop=(kc == KC - 1),
                            )
                        nc.vector.tensor_copy(
                            v_sb[:, sc * 4 + mm, :, 0:DEPTH],
                            ps[:].rearrange("p (h d) -> p h d", h=GH),
                        )

            # ---------- phase 2: attention + interleaved projection ----------
            with (
                tc.tile_pool(name="expp", bufs=3) as expp,
                tc.tile_pool(name="onp", bufs=2) as onp,
                tc.tile_pool(name="rbp", bufs=2) as rbp,
                tc.tile_pool(name="outp", bufs=3) as outp,
                tc.tile_pool(name="pss", bufs=6, space="PSUM") as pss,
                tc.tile_pool(name="pso", bufs=2, space="PSUM") as pso,
            ):
                NT = 128  # (sqc, hp, kb) flat iterations
                SKEW = 2  # scores run this many blocks ahead of attnV
                pend = {}
                oX_by_hp = {}
                rb_by_sqc = {}
                rt_by = {}
                outN_by = {}

                def head_block(t):
                    return t // 32, (t // 16) % 2, t % 16  # sqc, hp, kb

                def emit_chain(sqc, hp):
                    # one hp's denominators: spread over 32 partitions,
                    # reciprocal, write back bf16, broadcast into rb
                    base = sqc * 2048 + hp * 1024
                    rr = rbp.tile([32, 32], fp32, tag="rr", name="rr")
                    nc.sync.dma_start(
                        rr[:],
                        raw_dram[base : base + 1024].rearrange(
                            "(p x) -> p x", p=32
                        ),
                    )
                    rr2 = rbp.tile([32, 32], bf16, tag="rr2", name="rr2")
                    with nc.allow_low_precision(reason="bf16 denom recip"):
                        nc.vector.reciprocal(rr2[:], rr[:])
                    nc.sync.dma_start(
                        rec_dram[base : base + 1024].rearrange(
                            "(p x) -> p x", p=32
                        ),
                        rr2[:],
                    )
                    if hp == 0:
                        rb_by_sqc[sqc] = rbp.tile(
                            [128, 2, 512], bf16, tag="rb", name="rb"
                        )
                    rb = rb_by_sqc[sqc]
                    for a in range(2):
                        rec_v = rec_dram[
                            base + a * 512 : base + (a + 1) * 512
                        ].rearrange("(p h s) -> p h s", p=1, h=1)
                        nc.sync.dma_start(
                            rb[ts(a, 64), ts(hp, 1), :],
                            rec_v.to_broadcast([64, 1, 512]),
                        )

                def emit_norm(sqc, mm, eng=None):
                    # normalize into a fresh tile: in-place on outTs would
                    # push it past tile's writer limit and coarsen deps
                    if mm == 0:
                        outN_by[sqc] = onp.tile(
                            [128, 2, 512], bf16, tag="on", name="on"
                        )
                    m = sqc * 4 + mm
                    eng = eng or nc.gpsimd
                    with nc.allow_low_precision(reason="bf16 softmax norm"):
                        eng.tensor_mul(
                            outN_by[sqc][:, :, ts(mm, 128)],
                            outTs[:, :, ts(m, 128)],
                            rb_by_sqc[sqc][:, :, ts(mm, 128)],
                        )

                def emit_pp(sqc, mm):
                    m = sqc * 4 + mm
                    for nn in range(2):
                        # borrow a transient scores slot (the "o"
                        # accumulators live a whole hp block and would
                        # deadlock the PE queue)
                        pp = pss.tile([128, 512], fp32, tag="s", name="pp")
                        for hp in range(2):
                            nc.tensor.matmul(
                                pp[:],
                                outN_by[sqc][:, hp, ts(mm, 128)],
                                wo_sb[:, hp, ts(nn, 512)],
                                start=(hp == 0),
                                stop=(hp == 1),
                            )
                        ot = outp.tile([128, 512], fp16, tag="ot", name="ot")
                        if nn == 0:
                            nc.scalar.activation(ot[:], pp[:], Copy)
                        else:
                            nc.vector.tensor_copy(ot[:], pp[:])
                        nc.sync.dma_start(out[ts(m, 128), ts(nn, 512)], ot[:])

                for t in range(NT + SKEW):
                    if t < NT:
                        sqc, hp, kb = head_block(t)
                        if kb == 0:
                            oX_by_hp[t // 16] = [
                                pso.tile([65, 512], fp32, tag="o", name=f"o{a}")
                                for a in range(2)
                            ]
                        # scores for block t (SKEW ahead of attnV)
                        sps = [
                            pss.tile([128, 512], fp32, tag="s", name=f"s{a}")
                            for a in range(2)
                        ]
                        for a in range(2):
                            nc.tensor.matmul(
                                sps[a][:],
                                kT[ts(a, 64), hp, ts(kb, 128)],
                                qT[ts(a, 64), hp, ts(sqc, 512)],
                                start=True,
                                stop=True,
                            )
                        ex = expp.tile([128, 2, 512], bf16, tag="e", bufs=4, name="ex")
                        nc.scalar.activation(ex[:, 0, :], sps[0][:], Exp)
                        nc.vector.tensor_scalar(
                            ex[:, 1, :].bitcast(i16),
                            sps[1][:],
                            SCHR_A,
                            SCHR_B,
                            Alu.mult,
                            Alu.add,
                        )
                        pend[t] = ex
                        # denominator chains and the previous chunk's
                        # projection, spread across this chunk's iterations
                        # so no engine sees a burst
                        r = t % 32
                        if r == 24:
                            emit_chain(sqc, 0)
                        elif r == 4 and sqc > 0:
                            emit_chain(sqc - 1, 1)
                        if 0 < sqc < 3 and r in (10, 14, 18, 22, 26):
                            j = (r - 10) // 4
                            if j < 4:
                                emit_norm(sqc - 1, j)
                            if j > 0:
                                emit_pp(sqc - 1, j - 1)
                        if 0 < sqc < 3 and r == 30:
                            emit_pp(sqc - 1, 3)

                    tp = t - SKEW
                    if tp >= 0:
                        exp_t = pend.pop(tp)
                        psqc, php, pkb = head_block(tp)
                        oXp = oX_by_hp[tp // 16]
                        for a in range(2):
                            nc.tensor.matmul(
                                oXp[a][:],
                                v_sb[:, pkb, 2 * php + a, :],
                                exp_t[:, a, :],
                                start=(pkb == 0),
                                stop=(pkb == 15),
                            )
                        if pkb == 15:
                            # evacuate: stack pair members on partitions;
                            # denominator rows go through an SBUF scratch on
                            # ScalarE (DMA cannot read PSUM directly)
                            rt = rbp.tile(
                                [65, 2, 512], fp32, tag="rt", name="rt"
                            )
                            rt_by[(psqc, php)] = rt
                            for a in range(2):
                                nc.vector.tensor_copy(
                                    outTs[ts(a, 64), php, ts(psqc, 512)],
                                    oXp[a][0:64, :],
                                )
                                nc.scalar.activation(
                                    rt[64:65, a, :], oXp[a][64:65, :], Copy
                                )
                                ofs = psqc * 2048 + php * 1024 + a * 512
                                nc.sync.dma_start(
                                    raw_dram[ofs : ofs + 512].rearrange(
                                        "(p s) -> p s", p=1
                                    ),
                                    rt[64:65, a, :],
                                )

                # tail: normalize chunk 2 (ready), kick the last chunk's
                # hp1 chain, fill its DMA latency with chunk 2's projection
                # (emitting the chain after the norms keeps proj(2) off the
                # chain's cumulative DMA-semaphore thresholds), then chunk 3
                for mm in range(4):
                    emit_norm(2, mm, eng=nc.vector)
                emit_chain(3, 1)
                for mm in range(4):
                    emit_pp(2, mm)
                for mm in range(4):
                    emit_norm(3, mm, eng=nc.vector)
                    emit_pp(3, mm)

    nc.compile()
    return nc


def _get_nc():
    if "nc" not in _state:
        _state["nc"] = _build()
    return _state["nc"]


def _prep_core_inputs(inputs, Wq, bq, Wk, bk, Wv, bv, Wo, bo):
    bf = ml_dtypes.bfloat16
    in_maps = []
    for c in range(NCORES):
        b, g = divmod(c, 4)
        cols = slice(g * GD, (g + 1) * GD)
        bqk = np.zeros((128, 4), np.float32)
        bqk[:, 0:2] = bq[cols].reshape(2, 128).T
        bqk[:, 2:4] = bk[cols].reshape(2, 128).T
        m = {
            "xT": np.ascontiguousarray(inputs[b].T).astype(bf),
            "wq": np.ascontiguousarray(Wq[:, cols]).astype(bf),
            "wk": np.ascontiguousarray(Wk[:, cols]).astype(bf),
            "wv": np.ascontiguousarray(Wv[:, cols]).astype(bf),
            "wo": np.ascontiguousarray(Wo[cols, :]).astype(bf),
            "bqk": bqk,
            "bv": bv[cols].reshape(1, GD).astype(bf),
        }
        in_maps.append(m)
    return in_maps


def run(inputs, Wq, bq, Wk, bk, Wv, bv, Wo, bo, trace=False):
    from concourse.bass_utils import run_bass_kernel_spmd

    nc = _get_nc()
    in_maps = _prep_core_inputs(inputs, Wq, bq, Wk, bk, Wv, bv, Wo, bo)
    res = run_bass_kernel_spmd(
        nc, in_maps, core_ids=list(range(NCORES)), trace=trace
    )
    out = np.zeros((2, S, D), np.float32)
    for c in range(NCORES):
        out[c // 4] += res.results[c]["out"].astype(np.float32)
    out += np.asarray(bo, np.float32)
    return out, res


def kernel(inputs, Wq, bq, Wk, bk, Wv, bv, Wo, bo):
    out, _ = run(
        np.asarray(inputs, np.float32),
        np.asarray(Wq, np.float32), np.asarray(bq, np.float32),
        np.asarray(Wk, np.float32), np.asarray(bk, np.float32),
        np.asarray(Wv, np.float32), np.asarray(bv, np.float32),
        np.asarray(Wo, np.float32), np.asarray(bo, np.float32),
    )
    return out
